# revision 9
# baseline (speedup 1.0000x reference)
"""Pairwise-interaction kernel for Trainium2 (raw Bass), 8-core SPMD.

v6: same as v5b but supertile 0 is split into two half-group (GS=2)
loads and sweeps so the first store chunk is ready ~2.5us earlier.
See v5b docstring for the full design rationale.
"""

import numpy as np
import ml_dtypes

import concourse.bass as bass
from concourse import mybir
from concourse.bass_utils import run_bass_kernel_spmd

B, F, D = 16384, 26, 32
NCORES = 8
BC = B // NCORES           # 2048 samples per core
P = 128                    # SBUF partitions
G = 4                      # sample groups per supertile (consecutive rows)
GS = G // 2                # groups per half-sweep (supertile 0 only)
NTS = BC // (P * G)        # 4 supertiles per core
FD = F * D                 # 832
NPAIR = F * (F - 1) // 2   # 325
OD = NPAIR * D             # 10400

XB = NTS                   # all input supertiles resident at once
YB = 2                     # output supertile buffers

CHUNKS = [(0, 1), (1, 17), (17, 25)]
NCH = len(CHUNKS)

BF16 = mybir.dt.bfloat16
NP_BF16 = ml_dtypes.bfloat16


def _pair_off(i_lo):
    return sum(F - 1 - i for i in range(i_lo))


def _st_ord(t, c):
    # store ordinal (1-based) of chunk c of supertile t on the sync ring:
    # t=0 is stored as two half-group passes of NCH chunks each
    return 2 * NCH + NCH * (t - 1) + c + 1 if t >= 1 else NCH + c + 1


_nc_cache = None


def _build_nc():
    nc = bass.Bass()
    x = nc.declare_dram_parameter("x", [BC, FD], BF16, isOutput=False)
    y = nc.declare_dram_parameter("y", [BC, OD], BF16, isOutput=True)
    xv = x[:].rearrange("(t p g) m -> t p (g m)", p=P, g=G)
    yv = y[:].rearrange("(t p g) m -> t p g m", p=P, g=G)

    with (
        nc.sbuf_tensor([P, XB * G * FD], BF16) as xbuf,
        nc.sbuf_tensor([P, YB * G * OD], BF16) as ybuf,
        nc.semaphore("sem_ld") as sem_ld,
        nc.semaphore("sem_st") as sem_st,
        nc.semaphore("sem_tt") as sem_tt,
        nc.Block() as blk,
    ):
        xts = [xbuf[:, b * G * FD : (b + 1) * G * FD] for b in range(XB)]
        yts = [ybuf[:, b * G * OD : (b + 1) * G * OD] for b in range(YB)]

        def sweep(v, xt, yt, g_lo, g_hi):
            ng = g_hi - g_lo
            for i_lo, i_hi in CHUNKS:
                off = _pair_off(i_lo)
                for i in range(i_lo, i_hi):
                    nrep = F - 1 - i
                    in0 = (
                        xt[:, g_lo:g_hi, i * D : (i + 1) * D]
                        .unsqueeze(2)
                        .broadcast_to([P, ng, nrep, D])
                    )
                    in1 = xt[:, g_lo:g_hi, (i + 1) * D : FD].rearrange(
                        "p g (r d) -> p g r d", d=D
                    )
                    outap = yt[
                        :, g_lo:g_hi, off * D : (off + nrep) * D
                    ].rearrange("p g (r d) -> p g r d", d=D)
                    tt = nc.vector.tensor_mul(outap, in0, in1)
                    off += nrep
                tt.then_inc(sem_tt, 1)

        @blk.scalar
        def _(scalar):
            for h in range(2):
                scalar.dma_start(
                    xts[0][:, h * GS * FD : (h + 1) * GS * FD],
                    xv[0][:, h * GS * FD : (h + 1) * GS * FD],
                ).then_inc(sem_ld, 16)
            for t in range(1, NTS):
                scalar.dma_start(xts[t], xv[t]).then_inc(sem_ld, 16)

        @blk.sync
        def _(sync):
            yt0 = yts[0].rearrange("p (g m) -> p g m", g=G)
            for h in range(2):
                for c, (i_lo, i_hi) in enumerate(CHUNKS):
                    p_lo, p_hi = _pair_off(i_lo), _pair_off(i_hi)
                    st = sync.dma_start(
                        yv[0][:, h * GS : (h + 1) * GS, p_lo * D : p_hi * D],
                        yt0[:, h * GS : (h + 1) * GS, p_lo * D : p_hi * D],
                    )
                    st._wait_ge(sem_tt, NCH * h + c + 1)
                    st.then_inc(sem_st, 16)
            for t in range(1, NTS):
                yt = yts[t % YB].rearrange("p (g m) -> p g m", g=G)
                for c, (i_lo, i_hi) in enumerate(CHUNKS):
                    p_lo, p_hi = _pair_off(i_lo), _pair_off(i_hi)
                    st = sync.dma_start(
                        yv[t][:, :, p_lo * D : p_hi * D],
                        yt[:, :, p_lo * D : p_hi * D],
                    )
                    st._wait_ge(sem_tt, NCH * (t + 1) + c + 1)
                    st.then_inc(sem_st, 16)

        @blk.vector
        def _(v):
            xt0 = xts[0].rearrange("p (g m) -> p g m", g=G)
            yt0 = yts[0].rearrange("p (g m) -> p g m", g=G)
            for h in range(2):
                v.wait_ge(sem_ld, 16 * (h + 1))
                sweep(v, xt0, yt0, h * GS, (h + 1) * GS)
            for t in range(1, NTS):
                xt = xts[t].rearrange("p (g m) -> p g m", g=G)
                yt = yts[t % YB].rearrange("p (g m) -> p g m", g=G)
                v.wait_ge(sem_ld, 16 * (t + 2))
                for c, (i_lo, i_hi) in enumerate(CHUNKS):
                    if t >= YB:
                        v.wait_ge(sem_st, 16 * _st_ord(t - YB, c))
                    off = _pair_off(i_lo)
                    for i in range(i_lo, i_hi):
                        nrep = F - 1 - i
                        in0 = (
                            xt[:, :, i * D : (i + 1) * D]
                            .unsqueeze(2)
                            .broadcast_to([P, G, nrep, D])
                        )
                        in1 = xt[:, :, (i + 1) * D : FD].rearrange(
                            "p g (r d) -> p g r d", d=D
                        )
                        outap = yt[
                            :, :, off * D : (off + nrep) * D
                        ].rearrange("p g (r d) -> p g r d", d=D)
                        tt = nc.vector.tensor_mul(outap, in0, in1)
                        off += nrep
                    tt.then_inc(sem_tt, 1)

    return nc


def _make_in_maps(inputs: np.ndarray):
    x = np.asarray(inputs, dtype=np.float32).reshape(B, FD).astype(NP_BF16)
    shards = np.ascontiguousarray(x.reshape(NCORES, BC, FD))
    return [{"x": shards[c]} for c in range(NCORES)]


def kernel(inputs: np.ndarray) -> np.ndarray:
    global _nc_cache
    if _nc_cache is None:
        _nc_cache = _build_nc()
    nc = _nc_cache

    in_maps = _make_in_maps(inputs)
    res = run_bass_kernel_spmd(nc, in_maps, list(range(NCORES)))
    out = np.concatenate([res.results[c]["y"] for c in range(NCORES)], axis=0)
    return out.astype(np.float32).reshape(B, NPAIR, D)


# revision 10
# speedup vs baseline: 1.1203x; 1.1203x over previous
"""Pairwise-interaction kernel for Trainium2 (raw Bass), 8-core SPMD.

Computes out[b, p, :] = x[b, i(p), :] * x[b, j(p), :] for all pairs
(i < j) of the F=26 feature rows, p ordered row-major (i outer, j inner).

Sharding: data-parallel over the batch dim (16384 -> 8 x 2048), no
cross-device communication.

Design (f32 v1 baseline was ~221us local / 247us harness; this runs
~108-117us, at the HBM roofline for the mandatory traffic):
  * All tensors bf16: DVE 2x packing mode doubles tensor_tensor
    throughput vs f32 (the f32 kernel was vector-bound at ~203us busy)
    AND halves HBM traffic to 42.6MB stores + 3.4MB loads per core.
    The added rounding error (two input roundings + one output
    rounding, each <=2^-8: ~1.2% worst case, 1.07e-2 measured) is well
    inside the 2e-2 relative-error gate; f32<->bf16 conversion happens
    on the host.
  * Samples are interleaved G=4 per partition row (sample =
    t*P*G + p*G + g): every TT instruction covers all 4 groups through
    a [P, G, nrep, D] broadcast AP (amortizes the ~58-cycle DVE
    per-instruction bubble -> DVE ~95us busy), and each DMA descriptor
    row is a multi-KB contiguous DRAM run (4 consecutive samples per
    partition). ~5KB descriptor rows measured packet-overhead-bound at
    ~338 GB/s; with 10-17KB rows stores sustain ~420 GB/s, ~97% of the
    435 GB/s per-core SBUF-AXI fabric ceiling (the two NCs sharing an
    HBM stack cap the pair at ~716 GB/s, which is what binds when all
    8 phase-locked cores stream stores concurrently).
  * The exec floor is the store stream: first-chunk-ready + 42.6MB /
    sustained-rate. The 16 SDMA engines are shared by both HWDGE
    rings, so splitting stores across rings buys nothing; all stores
    ride the sync ring, and loads ride the scalar ring so a load never
    queues behind a multi-MB store (an earlier revision lost 23us to
    exactly that). Store chunks are pair-ranges sized tiny/huge/small
    (25/264/36 pairs) so the stream starts ~2us into the first sweep
    and the post-compute drain is only ~2.8us.
  * All NTS=4 input loads are issued up-front (XB=NTS buffers, 27KB of
    SBUF); supertile 0 is split into two half-group (GS=2) loads and
    sweeps so the first store chunk is ready ~2.5us earlier.

Raw-Bass sync scheme (one semaphore wait per instruction; extra
ordering uses standalone wait_ge ops on the engine queue):
  sem_ld (+16 per load DMA, scalar ring)
  sem_st (+16 per store DMA, sync ring)
  sem_tt (+1 by the last TT of each chunk, vector engine)
"""

import numpy as np
import ml_dtypes

import concourse.bass as bass
from concourse import mybir
from concourse.bass_utils import run_bass_kernel_spmd

B, F, D = 16384, 26, 32
NCORES = 8
BC = B // NCORES           # 2048 samples per core
P = 128                    # SBUF partitions
G = 4                      # sample groups per supertile (consecutive rows)
GS = G // 2                # groups per half-sweep (supertile 0 only)
NTS = BC // (P * G)        # 4 supertiles per core
FD = F * D                 # 832
NPAIR = F * (F - 1) // 2   # 325
OD = NPAIR * D             # 10400

XB = NTS                   # all input supertiles resident at once
YB = 2                     # output supertile buffers

CHUNKS = [(0, 1), (1, 17), (17, 25)]
NCH = len(CHUNKS)

BF16 = mybir.dt.bfloat16
NP_BF16 = ml_dtypes.bfloat16


def _pair_off(i_lo):
    return sum(F - 1 - i for i in range(i_lo))


def _st_ord(t, c):
    # store ordinal (1-based) of chunk c of supertile t on the sync ring:
    # t=0 is stored as two half-group passes of NCH chunks each
    return 2 * NCH + NCH * (t - 1) + c + 1 if t >= 1 else NCH + c + 1


_nc_cache = None


def _build_nc():
    nc = bass.Bass()
    x = nc.declare_dram_parameter("x", [BC, FD], BF16, isOutput=False)
    y = nc.declare_dram_parameter("y", [BC, OD], BF16, isOutput=True)
    xv = x[:].rearrange("(t p g) m -> t p (g m)", p=P, g=G)
    yv = y[:].rearrange("(t p g) m -> t p g m", p=P, g=G)

    with (
        nc.sbuf_tensor([P, XB * G * FD], BF16) as xbuf,
        nc.sbuf_tensor([P, YB * G * OD], BF16) as ybuf,
        nc.semaphore("sem_ld") as sem_ld,
        nc.semaphore("sem_st") as sem_st,
        nc.semaphore("sem_tt") as sem_tt,
        nc.Block() as blk,
    ):
        xts = [xbuf[:, b * G * FD : (b + 1) * G * FD] for b in range(XB)]
        yts = [ybuf[:, b * G * OD : (b + 1) * G * OD] for b in range(YB)]

        def sweep(v, xt, yt, g_lo, g_hi):
            ng = g_hi - g_lo
            for i_lo, i_hi in CHUNKS:
                off = _pair_off(i_lo)
                for i in range(i_lo, i_hi):
                    nrep = F - 1 - i
                    in0 = (
                        xt[:, g_lo:g_hi, i * D : (i + 1) * D]
                        .unsqueeze(2)
                        .broadcast_to([P, ng, nrep, D])
                    )
                    in1 = xt[:, g_lo:g_hi, (i + 1) * D : FD].rearrange(
                        "p g (r d) -> p g r d", d=D
                    )
                    outap = yt[
                        :, g_lo:g_hi, off * D : (off + nrep) * D
                    ].rearrange("p g (r d) -> p g r d", d=D)
                    tt = nc.vector.tensor_mul(outap, in0, in1)
                    off += nrep
                tt.then_inc(sem_tt, 1)

        @blk.scalar
        def _(scalar):
            for h in range(2):
                scalar.dma_start(
                    xts[0][:, h * GS * FD : (h + 1) * GS * FD],
                    xv[0][:, h * GS * FD : (h + 1) * GS * FD],
                ).then_inc(sem_ld, 16)
            for t in range(1, NTS):
                scalar.dma_start(xts[t], xv[t]).then_inc(sem_ld, 16)

        @blk.sync
        def _(sync):
            yt0 = yts[0].rearrange("p (g m) -> p g m", g=G)
            for h in range(2):
                for c, (i_lo, i_hi) in enumerate(CHUNKS):
                    p_lo, p_hi = _pair_off(i_lo), _pair_off(i_hi)
                    st = sync.dma_start(
                        yv[0][:, h * GS : (h + 1) * GS, p_lo * D : p_hi * D],
                        yt0[:, h * GS : (h + 1) * GS, p_lo * D : p_hi * D],
                    )
                    st._wait_ge(sem_tt, NCH * h + c + 1)
                    st.then_inc(sem_st, 16)
            for t in range(1, NTS):
                yt = yts[t % YB].rearrange("p (g m) -> p g m", g=G)
                for c, (i_lo, i_hi) in enumerate(CHUNKS):
                    p_lo, p_hi = _pair_off(i_lo), _pair_off(i_hi)
                    st = sync.dma_start(
                        yv[t][:, :, p_lo * D : p_hi * D],
                        yt[:, :, p_lo * D : p_hi * D],
                    )
                    st._wait_ge(sem_tt, NCH * (t + 1) + c + 1)
                    st.then_inc(sem_st, 16)

        @blk.vector
        def _(v):
            xt0 = xts[0].rearrange("p (g m) -> p g m", g=G)
            yt0 = yts[0].rearrange("p (g m) -> p g m", g=G)
            for h in range(2):
                v.wait_ge(sem_ld, 16 * (h + 1))
                sweep(v, xt0, yt0, h * GS, (h + 1) * GS)
            for t in range(1, NTS):
                xt = xts[t].rearrange("p (g m) -> p g m", g=G)
                yt = yts[t % YB].rearrange("p (g m) -> p g m", g=G)
                v.wait_ge(sem_ld, 16 * (t + 2))
                for c, (i_lo, i_hi) in enumerate(CHUNKS):
                    if t >= YB:
                        v.wait_ge(sem_st, 16 * _st_ord(t - YB, c))
                    off = _pair_off(i_lo)
                    for i in range(i_lo, i_hi):
                        nrep = F - 1 - i
                        in0 = (
                            xt[:, :, i * D : (i + 1) * D]
                            .unsqueeze(2)
                            .broadcast_to([P, G, nrep, D])
                        )
                        in1 = xt[:, :, (i + 1) * D : FD].rearrange(
                            "p g (r d) -> p g r d", d=D
                        )
                        outap = yt[
                            :, :, off * D : (off + nrep) * D
                        ].rearrange("p g (r d) -> p g r d", d=D)
                        tt = nc.vector.tensor_mul(outap, in0, in1)
                        off += nrep
                    tt.then_inc(sem_tt, 1)

    return nc


def _make_in_maps(inputs: np.ndarray):
    x = np.asarray(inputs, dtype=np.float32).reshape(B, FD).astype(NP_BF16)
    shards = np.ascontiguousarray(x.reshape(NCORES, BC, FD))
    return [{"x": shards[c]} for c in range(NCORES)]


def kernel(inputs: np.ndarray) -> np.ndarray:
    global _nc_cache
    if _nc_cache is None:
        _nc_cache = _build_nc()
    nc = _nc_cache

    in_maps = _make_in_maps(inputs)
    res = run_bass_kernel_spmd(nc, in_maps, list(range(NCORES)))
    out = np.concatenate([res.results[c]["y"] for c in range(NCORES)], axis=0)
    return out.astype(np.float32).reshape(B, NPAIR, D)
